# revision 2
# baseline (speedup 1.0000x reference)
"""Trainium2 Bass kernel v2 for nn_AttentionLayer (masked attention pooling).

Math per batch row b (after folding H):
    v_b     = tanh(qe_b @ Wq.T + bq).reshape(D, H) @ Wr.T        # (D,)
    s_b     = item_b @ v_b                                        # (N,)
    att     = exp(s_b) * mask_b        (softmax shift-invariant; f32 exp safe)
    denom   = sum(att); if denom < emax*1e-7: denom += emax       # ref guard
    out_b   = item_b.T @ (att / denom)                            # (D,)

Architecture (vs v1 which ran everything on DVE at 2x ~ 123us busy):
  * item is host-converted to bf16 AND host-transposed to itemT[d, b*N+n]
    (d on partitions).  Halves DMA (26MB -> 13MB) and kills the on-device
    f32->bf16 converts that used 45us of ACT.
  * scores run on the TENSOR engine: per row, item_b^T is the stationary
    operand (2 LDWEIGHTS: 128-col FWL + 72-col) and v_b a 1-column moving
    operand -> scores land densely as PSUM[n, b] columns.  ~30us PE.
  * exp on ACT straight from PSUM (f32 in, critical for softmax accuracy),
    bf16 out; att transposed to [b, n] via PE-transpose; mask+guard+1/denom
    normalization all as per-partition-scalar DVE ops.
  * weights flattened to one [1, 32*200] row per 32-row chunk (striped
    across partitions, cheap contiguous DMA) and broadcast-read by DVE
    (partition-stride-0) for the weighted sum:
      mult [128, 6400] bf16 @2x + ragged halving tree over n.
    DVE does ONLY this (~55us) - the new critical path.
  * everything pipelines at 32-row-chunk granularity behind the item DMA.
"""

import sys

if "/opt/trn_rl_repo" not in sys.path:
    sys.path.insert(0, "/opt/trn_rl_repo")

from contextlib import ExitStack

import numpy as np

import concourse.bass as bass
import concourse.bacc as bacc
import concourse.tile as tile
from concourse import mybir

B, N, D, H = 2048, 200, 128, 8
NCORES = 8
BS = B // NCORES          # 256 batch rows per core
DH = D * H                # 1024
BCH = 32                  # rows per pipeline chunk
NCHK = BS // BCH          # 8 chunks
CW = BCH * N              # 6400 itemT cols per chunk

F32 = mybir.dt.float32
BF16 = mybir.dt.bfloat16
AX = mybir.AxisListType
OP = mybir.AluOpType
ACT = mybir.ActivationFunctionType

_CACHE = {}


def build_module() -> bass.Bass:
    nc = bacc.Bacc("TRN2", target_bir_lowering=False)

    itemT_in = nc.declare_dram_parameter("itemT", [D, BS * N], BF16, isOutput=False)
    qeT_in = nc.declare_dram_parameter("qeT", [D, BS], BF16, isOutput=False)
    wqT_in = nc.declare_dram_parameter("wqT", [D, DH], BF16, isOutput=False)
    bqT_in = nc.declare_dram_parameter("bqT", [D, H], F32, isOutput=False)
    wsel_in = nc.declare_dram_parameter("wsel", [D, DH], BF16, isOutput=False)
    maskb_in = nc.declare_dram_parameter("maskb", [128, 2 * N], BF16, isOutput=False)
    ident_in = nc.declare_dram_parameter("ident", [D, D], BF16, isOutput=False)
    outT = nc.declare_dram_parameter("outT", [D, BS], F32, isOutput=True)
    # DRAM scratch for flat weight rows (SBUF->SBUF broadcast is illegal;
    # DRAM-source broadcast-read is the supported replication path)
    wflat_d = nc.dram_tensor("wflat_d", [NCHK, CW], BF16, kind="Internal")

    with tile.TileContext(nc) as tc, ExitStack() as ctx:
        const = ctx.enter_context(tc.tile_pool(name="const", bufs=1))
        items = ctx.enter_context(tc.tile_pool(name="items", bufs=1))
        work = ctx.enter_context(tc.tile_pool(name="work", bufs=3))
        tmps = ctx.enter_context(tc.tile_pool(name="tmps", bufs=2))
        small = ctx.enter_context(tc.tile_pool(name="small", bufs=4))
        proj = ctx.enter_context(tc.tile_pool(name="proj", bufs=1, space="PSUM"))
        psum = ctx.enter_context(tc.tile_pool(name="psum", bufs=2, space="PSUM"))
        pst = ctx.enter_context(tc.tile_pool(name="pst", bufs=2, space="PSUM"))

        # ---- small inputs first (FIFO queues: don't let them trail item)
        qeT = const.tile([D, BS], BF16)
        nc.sync.dma_start(qeT[:], qeT_in[:])
        wqT = const.tile([D, DH], BF16)
        nc.sync.dma_start(wqT[:], wqT_in[:])
        wsel = const.tile([D, DH], BF16)
        nc.sync.dma_start(wsel[:], wsel_in[:])
        bqT = const.tile([D, H], F32)
        nc.sync.dma_start(bqT[:], bqT_in[:])
        ident = const.tile([D, D], BF16)
        nc.sync.dma_start(ident[:], ident_in[:])
        # mask folded to [128, 2N]: rows 128:256 live in cols N:2N
        maskb = const.tile([128, 2 * N], BF16)
        nc.sync.dma_start(maskb[:], maskb_in[:])

        # ---- item stream: 2 DMAs per 32-row chunk
        itemT = items.tile([D, BS * N], BF16)
        for k in range(NCHK):
            cs = k * CW
            nc.sync.dma_start(itemT[:, cs:cs + CW // 2], itemT_in[:, cs:cs + CW // 2])
            nc.sync.dma_start(
                itemT[:, cs + CW // 2:cs + CW], itemT_in[:, cs + CW // 2:cs + CW])

        # ---- query projection: pqT[dh, b] = tanh(Wq @ qe.T + bq), chunked
        # over dh in 8 blocks of 128 partitions; then vT[d, b] via the
        # host-built block-diagonal selection weights (wsel embeds Wr).
        pqT = work.tile([D, H * BS], BF16, tag="pqT")
        for c in range(H):
            pq_ps = proj.tile([D, BS], F32, tag="pq")
            nc.tensor.matmul(
                pq_ps[:], wqT[:, c * D:(c + 1) * D], qeT[:], start=True, stop=True)
            nc.scalar.activation(
                pqT[:, c * BS:(c + 1) * BS], pq_ps[:], ACT.Tanh,
                bias=bqT[:, c:c + 1])
        vt_ps = proj.tile([D, BS], F32, tag="vt")
        for c in range(H):
            nc.tensor.matmul(
                vt_ps[:], wsel[:, c * D:(c + 1) * D], pqT[:, c * BS:(c + 1) * BS],
                start=(c == 0), stop=(c == H - 1))
        vTb = work.tile([D, BS], BF16, tag="vTb")
        nc.scalar.copy(vTb[:], vt_ps[:])



        for k in range(NCHK):
            cs = k * CW

            # ---- scores on PE: stationary item_b^T (LDW), moving v col
            psA = psum.tile([128, BCH], F32, tag="psA")
            psB = psum.tile([72, BCH], F32, tag="psB")
            for j in range(BCH):
                b = k * BCH + j
                col = b * N
                nc.tensor.matmul(
                    psA[:, j:j + 1], itemT[:, col:col + 128], vTb[:, b:b + 1],
                    start=True, stop=True)
                nc.tensor.matmul(
                    psB[:, j:j + 1], itemT[:, col + 128:col + N], vTb[:, b:b + 1],
                    start=True, stop=True)

            # ---- exp from PSUM (f32 in -> bf16 out), then PE-transpose
            e_lo = work.tile([128, BCH], BF16, tag="elo")
            nc.scalar.activation(e_lo[:], psA[:], ACT.Exp)
            e_hi = work.tile([72, BCH], BF16, tag="ehi")
            nc.scalar.activation(e_hi[:], psB[:], ACT.Exp)
            psT = pst.tile([BCH, N], BF16, tag="psT")
            nc.tensor.transpose(psT[:, 0:128], e_lo[:], ident[:])
            nc.tensor.transpose(psT[:, 128:N], e_hi[:], ident[0:72, 0:72])

            # ---- mask, guard, normalize (all [BCH, *] per-partition ops)
            att = small.tile([BCH, N], BF16, tag="att")
            mrow = (k % 4) * BCH
            mcol = (k // 4) * N
            nc.vector.tensor_tensor(
                att[:], psT[:], maskb[mrow:mrow + BCH, mcol:mcol + N], OP.mult)
            emax = small.tile([BCH, 1], F32, tag="emax")
            nc.vector.tensor_reduce(emax[:], psT[:], axis=AX.X, op=OP.max)
            den = small.tile([BCH, 1], F32, tag="den")
            nc.vector.tensor_reduce(den[:], att[:], axis=AX.X, op=OP.add)
            thr = small.tile([BCH, 1], F32, tag="thr")
            nc.vector.tensor_scalar(thr[:], emax[:], 1e-7, None, OP.mult)
            corr = small.tile([BCH, 1], F32, tag="corr")
            nc.vector.scalar_tensor_tensor(
                corr[:], den[:], thr[:], emax[:], op0=OP.is_lt, op1=OP.mult)
            nc.vector.tensor_tensor(den[:], den[:], corr[:], OP.add)
            inv = small.tile([BCH, 1], F32, tag="inv")
            nc.vector.reciprocal(inv[:], den[:])
            wgt = small.tile([BCH, N], BF16, tag="wgt")
            nc.vector.tensor_scalar(wgt[:], att[:], inv[:], None, OP.mult)

            # ---- flatten weights to a DRAM row, then broadcast-read them
            # back replicated across all 128 partitions (no compute engine)
            nc.sync.dma_start(
                wflat_d[k:k + 1, :].rearrange("o (b n) -> (o b) n", b=BCH), wgt[:])
            wrep = tmps.tile([D, CW], BF16, tag="wrep")
            nc.sync.dma_start(wrep[:], wflat_d[k:k + 1, :].to_broadcast([D, CW]))

            # ---- weighted sum on DVE: mult then ragged halving tree over n
            tmp = tmps.tile([D, CW], BF16, tag="tmp")
            nc.vector.tensor_tensor(tmp[:], itemT[:, cs:cs + CW], wrep[:], OP.mult)
            t3 = tmp[:].rearrange("p (b n) -> p b n", b=BCH)
            w = N
            while w > 2:
                lo = w // 2
                nc.vector.tensor_tensor(
                    t3[:, :, 0:lo], t3[:, :, 0:lo], t3[:, :, w - lo:w], OP.add)
                w = w - lo
            o_sb = work.tile([D, BCH], F32, tag="osb")
            nc.vector.tensor_tensor(
                o_sb[:], t3[:, :, 0], t3[:, :, 1], OP.add)
            nc.sync.dma_start(outT[:, k * BCH:(k + 1) * BCH], o_sb[:])

    nc.compile()
    return nc


def _get_module() -> bass.Bass:
    if "nc" not in _CACHE:
        _CACHE["nc"] = build_module()
    return _CACHE["nc"]


def make_in_maps(item_embedding, query_embedding, mask, Wq, bq, Wr):
    import ml_dtypes

    bf16 = ml_dtypes.bfloat16
    item = np.asarray(item_embedding, dtype=np.float32)
    qe = np.asarray(query_embedding, dtype=np.float32)
    mk = np.asarray(mask).reshape(B, N)
    wq = np.asarray(Wq, dtype=np.float32)
    bqv = np.asarray(bq, dtype=np.float32).reshape(DH)
    wr = np.asarray(Wr, dtype=np.float32).reshape(H)

    wqT = np.ascontiguousarray(wq.T.astype(bf16))            # [D, DH]
    bqT = np.ascontiguousarray(bqv.reshape(D, H))            # bq[dh]=bqT[d,h]; chunk c bias = bq[128c+j]
    # bqT layout: chunk c of 128 dh's -> partition j gets bq[c*128 + j]
    bqT = np.ascontiguousarray(bqv.reshape(H, D).T)          # [D, H]: [j, c] = bq[c*128+j]
    # wsel: chunk c: [k, d'] = Wr[(128c+k) % 8] if (128c+k)//8 == d' else 0
    wsel = np.zeros((D, DH), dtype=np.float32)
    for c in range(H):
        dh = np.arange(128) + 128 * c
        wsel[np.arange(128), c * D + dh // 8] = wr[dh % 8]
    wsel = np.ascontiguousarray(wsel.astype(bf16))
    ident = np.eye(D, dtype=np.float32).astype(bf16)

    in_maps = []
    for i in range(NCORES):
        r = slice(i * BS, (i + 1) * BS)
        itc = np.ascontiguousarray(
            item[r].astype(bf16).transpose(2, 0, 1).reshape(D, BS * N))
        in_maps.append({
            "itemT": itc,
            "qeT": np.ascontiguousarray(qe[r].T.astype(bf16)),
            "wqT": wqT,
            "bqT": bqT,
            "wsel": wsel,
            "maskb": np.ascontiguousarray(
                np.concatenate([mk[r][0:128], mk[r][128:BS]], axis=1).astype(bf16)),
            "ident": ident,
        })
    return in_maps


def kernel(item_embedding, query_embedding, mask, Wq, bq, Wr):
    from concourse.bass_utils import run_bass_kernel_spmd

    nc = _get_module()
    in_maps = make_in_maps(item_embedding, query_embedding, mask, Wq, bq, Wr)
    last_err = None
    for attempt in range(3):
        try:
            res = run_bass_kernel_spmd(
                nc, in_maps, core_ids=list(range(NCORES)),
                **_CACHE.get("run_kwargs", {})
            )
            break
        except Exception as e:  # transient NRT_EXEC_UNIT_UNRECOVERABLE flakes
            last_err = e
    else:
        raise last_err
    _CACHE["last_results"] = res
    return np.concatenate(
        [res.results[i]["outT"].T for i in range(NCORES)], axis=0)


# revision 3
# speedup vs baseline: 1.0019x; 1.0019x over previous
"""Trainium2 Bass kernel v2 for nn_AttentionLayer (masked attention pooling).

Math per batch row b (after folding H):
    v_b     = tanh(qe_b @ Wq.T + bq).reshape(D, H) @ Wr.T        # (D,)
    s_b     = item_b @ v_b                                        # (N,)
    att     = exp(s_b) * mask_b        (softmax shift-invariant; f32 exp safe)
    denom   = sum(att); if denom < emax*1e-7: denom += emax       # ref guard
    out_b   = item_b.T @ (att / denom)                            # (D,)

Architecture (vs v1 which ran everything on DVE at 2x ~ 123us busy):
  * item is host-converted to bf16 AND host-transposed to itemT[d, b*N+n]
    (d on partitions).  Halves DMA (26MB -> 13MB) and kills the on-device
    f32->bf16 converts that used 45us of ACT.
  * scores run on the TENSOR engine: per row, item_b^T is the stationary
    operand (2 LDWEIGHTS: 128-col FWL + 72-col) and v_b a 1-column moving
    operand -> scores land densely as PSUM[n, b] columns.  ~30us PE.
  * exp on ACT straight from PSUM (f32 in, critical for softmax accuracy),
    bf16 out; att transposed to [b, n] via PE-transpose; mask+guard+1/denom
    normalization all as per-partition-scalar DVE ops.
  * weights flattened to one [1, 32*200] row per 32-row chunk (striped
    across partitions, cheap contiguous DMA) and broadcast-read by DVE
    (partition-stride-0) for the weighted sum:
      mult [128, 6400] bf16 @2x + ragged halving tree over n.
    DVE does ONLY this (~55us) - the new critical path.
  * everything pipelines at 32-row-chunk granularity behind the item DMA.
"""

import sys

if "/opt/trn_rl_repo" not in sys.path:
    sys.path.insert(0, "/opt/trn_rl_repo")

from contextlib import ExitStack

import numpy as np

import concourse.bass as bass
import concourse.bacc as bacc
import concourse.tile as tile
from concourse import mybir

B, N, D, H = 2048, 200, 128, 8
NCORES = 8
BS = B // NCORES          # 256 batch rows per core
DH = D * H                # 1024
BCH = 32                  # rows per pipeline chunk
NCHK = BS // BCH          # 8 chunks
CW = BCH * N              # 6400 itemT cols per chunk

F32 = mybir.dt.float32
BF16 = mybir.dt.bfloat16
AX = mybir.AxisListType
OP = mybir.AluOpType
ACT = mybir.ActivationFunctionType

_CACHE = {}


def build_module() -> bass.Bass:
    nc = bacc.Bacc("TRN2", target_bir_lowering=False)

    itemT_in = nc.declare_dram_parameter("itemT", [D, BS * N], BF16, isOutput=False)
    qeT_in = nc.declare_dram_parameter("qeT", [D, BS], BF16, isOutput=False)
    wqT_in = nc.declare_dram_parameter("wqT", [D, DH], BF16, isOutput=False)
    bqT_in = nc.declare_dram_parameter("bqT", [D, H], F32, isOutput=False)
    wsel_in = nc.declare_dram_parameter("wsel", [D, DH], BF16, isOutput=False)
    maskb_in = nc.declare_dram_parameter("maskb", [128, 2 * N], BF16, isOutput=False)
    ident_in = nc.declare_dram_parameter("ident", [D, D], BF16, isOutput=False)
    outT = nc.declare_dram_parameter("outT", [D, BS], F32, isOutput=True)
    # DRAM scratch for flat weight rows (SBUF->SBUF broadcast is illegal;
    # DRAM-source broadcast-read is the supported replication path)
    wflat_d = nc.dram_tensor("wflat_d", [1, BS * N], BF16, kind="Internal")

    with tile.TileContext(nc) as tc, ExitStack() as ctx:
        const = ctx.enter_context(tc.tile_pool(name="const", bufs=1))
        items = ctx.enter_context(tc.tile_pool(name="items", bufs=1))
        work = ctx.enter_context(tc.tile_pool(name="work", bufs=3))
        tmps = ctx.enter_context(tc.tile_pool(name="tmps", bufs=2))
        small = ctx.enter_context(tc.tile_pool(name="small", bufs=4))
        proj = ctx.enter_context(tc.tile_pool(name="proj", bufs=1, space="PSUM"))
        psum = ctx.enter_context(tc.tile_pool(name="psum", bufs=2, space="PSUM"))
        pst = ctx.enter_context(tc.tile_pool(name="pst", bufs=2, space="PSUM"))

        # ---- small inputs first (FIFO queues: don't let them trail item)
        qeT = const.tile([D, BS], BF16)
        nc.sync.dma_start(qeT[:], qeT_in[:])
        wqT = const.tile([D, DH], BF16)
        nc.sync.dma_start(wqT[:], wqT_in[:])
        wsel = const.tile([D, DH], BF16)
        nc.sync.dma_start(wsel[:], wsel_in[:])
        bqT = const.tile([D, H], F32)
        nc.sync.dma_start(bqT[:], bqT_in[:])
        ident = const.tile([D, D], BF16)
        nc.sync.dma_start(ident[:], ident_in[:])
        # mask folded to [128, 2N]: rows 128:256 live in cols N:2N
        maskb = const.tile([128, 2 * N], BF16)
        nc.sync.dma_start(maskb[:], maskb_in[:])

        # ---- item stream: 2 DMAs per 32-row chunk
        itemT = items.tile([D, BS * N], BF16)
        for k in range(NCHK):
            cs = k * CW
            nc.sync.dma_start(itemT[:, cs:cs + CW // 2], itemT_in[:, cs:cs + CW // 2])
            nc.sync.dma_start(
                itemT[:, cs + CW // 2:cs + CW], itemT_in[:, cs + CW // 2:cs + CW])

        # ---- query projection: pqT[dh, b] = tanh(Wq @ qe.T + bq), chunked
        # over dh in 8 blocks of 128 partitions; then vT[d, b] via the
        # host-built block-diagonal selection weights (wsel embeds Wr).
        pqT = work.tile([D, H * BS], BF16, tag="pqT")
        for c in range(H):
            pq_ps = proj.tile([D, BS], F32, tag="pq")
            nc.tensor.matmul(
                pq_ps[:], wqT[:, c * D:(c + 1) * D], qeT[:], start=True, stop=True)
            nc.scalar.activation(
                pqT[:, c * BS:(c + 1) * BS], pq_ps[:], ACT.Tanh,
                bias=bqT[:, c:c + 1])
        vt_ps = proj.tile([D, BS], F32, tag="vt")
        for c in range(H):
            nc.tensor.matmul(
                vt_ps[:], wsel[:, c * D:(c + 1) * D], pqT[:, c * BS:(c + 1) * BS],
                start=(c == 0), stop=(c == H - 1))
        vTb = work.tile([D, BS], BF16, tag="vTb")
        nc.scalar.copy(vTb[:], vt_ps[:])



        for k in range(NCHK):
            cs = k * CW

            # ---- scores on PE: stationary item_b^T (LDW), moving v col
            psA = psum.tile([128, BCH], F32, tag="psA")
            psB = psum.tile([72, BCH], F32, tag="psB")
            for j in range(BCH):
                b = k * BCH + j
                col = b * N
                nc.tensor.matmul(
                    psA[:, j:j + 1], itemT[:, col:col + 128], vTb[:, b:b + 1],
                    start=True, stop=True)
                nc.tensor.matmul(
                    psB[:, j:j + 1], itemT[:, col + 128:col + N], vTb[:, b:b + 1],
                    start=True, stop=True)

            # ---- exp from PSUM (f32 in -> bf16 out), then PE-transpose
            e_lo = work.tile([128, BCH], BF16, tag="elo")
            nc.scalar.activation(e_lo[:], psA[:], ACT.Exp)
            e_hi = work.tile([72, BCH], BF16, tag="ehi")
            nc.scalar.activation(e_hi[:], psB[:], ACT.Exp)
            psT = pst.tile([BCH, N], BF16, tag="psT")
            nc.tensor.transpose(psT[:, 0:128], e_lo[:], ident[:])
            nc.tensor.transpose(psT[:, 128:N], e_hi[:], ident[0:72, 0:72])

            # ---- mask, guard, normalize (all [BCH, *] per-partition ops)
            att = small.tile([BCH, N], BF16, tag="att")
            mrow = (k % 4) * BCH
            mcol = (k // 4) * N
            nc.vector.tensor_tensor(
                att[:], psT[:], maskb[mrow:mrow + BCH, mcol:mcol + N], OP.mult)
            emax = small.tile([BCH, 1], F32, tag="emax")
            nc.vector.tensor_reduce(emax[:], psT[:], axis=AX.X, op=OP.max)
            den = small.tile([BCH, 1], F32, tag="den")
            nc.vector.tensor_reduce(den[:], att[:], axis=AX.X, op=OP.add)
            thr = small.tile([BCH, 1], F32, tag="thr")
            nc.vector.tensor_scalar(thr[:], emax[:], 1e-7, None, OP.mult)
            corr = small.tile([BCH, 1], F32, tag="corr")
            nc.vector.scalar_tensor_tensor(
                corr[:], den[:], thr[:], emax[:], op0=OP.is_lt, op1=OP.mult)
            nc.vector.tensor_tensor(den[:], den[:], corr[:], OP.add)
            inv = small.tile([BCH, 1], F32, tag="inv")
            nc.vector.reciprocal(inv[:], den[:])
            wgt = small.tile([BCH, N], BF16, tag="wgt")
            nc.vector.tensor_scalar(wgt[:], att[:], inv[:], None, OP.mult)

            # ---- flatten weights to a DRAM row, then broadcast-read them
            # back replicated across all 128 partitions (no compute engine).
            # chunk 0 runs in two 16-row waves so the first DVE multiply
            # starts ~2.5us earlier during the pipeline ramp.
            wrep = tmps.tile([D, CW], BF16, tag="wrep")
            tmp = tmps.tile([D, CW], BF16, tag="tmp")
            o_sb = work.tile([D, BCH], F32, tag="osb")
            waves = ((0, 16), (16, 16)) if k == 0 else ((0, BCH),)
            for w0, ww in waves:
                fs = k * CW + w0 * N
                fw = ww * N
                fr = wflat_d[0:1, fs:fs + fw]
                nc.sync.dma_start(
                    fr.rearrange("o (b n) -> (o b) n", b=ww), wgt[w0:w0 + ww, :])
                nc.sync.dma_start(
                    wrep[:, w0 * N:w0 * N + fw], fr.to_broadcast([D, fw]))

                # weighted sum on DVE: mult then ragged halving tree over n
                ts = slice(w0 * N, w0 * N + fw)
                nc.vector.tensor_tensor(
                    tmp[:, ts], itemT[:, fs:fs + fw], wrep[:, ts], OP.mult)
                t3 = tmp[:, ts].rearrange("p (b n) -> p b n", b=ww)
                w = N
                while w > 2:
                    lo = w // 2
                    nc.vector.tensor_tensor(
                        t3[:, :, 0:lo], t3[:, :, 0:lo], t3[:, :, w - lo:w],
                        OP.add)
                    w = w - lo
                nc.vector.tensor_tensor(
                    o_sb[:, w0:w0 + ww], t3[:, :, 0], t3[:, :, 1], OP.add)
            nc.sync.dma_start(outT[:, k * BCH:(k + 1) * BCH], o_sb[:])

    nc.compile()
    return nc


def _get_module() -> bass.Bass:
    if "nc" not in _CACHE:
        _CACHE["nc"] = build_module()
    return _CACHE["nc"]


def make_in_maps(item_embedding, query_embedding, mask, Wq, bq, Wr):
    import ml_dtypes

    bf16 = ml_dtypes.bfloat16
    item = np.asarray(item_embedding, dtype=np.float32)
    qe = np.asarray(query_embedding, dtype=np.float32)
    mk = np.asarray(mask).reshape(B, N)
    wq = np.asarray(Wq, dtype=np.float32)
    bqv = np.asarray(bq, dtype=np.float32).reshape(DH)
    wr = np.asarray(Wr, dtype=np.float32).reshape(H)

    wqT = np.ascontiguousarray(wq.T.astype(bf16))            # [D, DH]
    bqT = np.ascontiguousarray(bqv.reshape(D, H))            # bq[dh]=bqT[d,h]; chunk c bias = bq[128c+j]
    # bqT layout: chunk c of 128 dh's -> partition j gets bq[c*128 + j]
    bqT = np.ascontiguousarray(bqv.reshape(H, D).T)          # [D, H]: [j, c] = bq[c*128+j]
    # wsel: chunk c: [k, d'] = Wr[(128c+k) % 8] if (128c+k)//8 == d' else 0
    wsel = np.zeros((D, DH), dtype=np.float32)
    for c in range(H):
        dh = np.arange(128) + 128 * c
        wsel[np.arange(128), c * D + dh // 8] = wr[dh % 8]
    wsel = np.ascontiguousarray(wsel.astype(bf16))
    ident = np.eye(D, dtype=np.float32).astype(bf16)

    in_maps = []
    for i in range(NCORES):
        r = slice(i * BS, (i + 1) * BS)
        itc = np.ascontiguousarray(
            item[r].astype(bf16).transpose(2, 0, 1).reshape(D, BS * N))
        in_maps.append({
            "itemT": itc,
            "qeT": np.ascontiguousarray(qe[r].T.astype(bf16)),
            "wqT": wqT,
            "bqT": bqT,
            "wsel": wsel,
            "maskb": np.ascontiguousarray(
                np.concatenate([mk[r][0:128], mk[r][128:BS]], axis=1).astype(bf16)),
            "ident": ident,
        })
    return in_maps


def kernel(item_embedding, query_embedding, mask, Wq, bq, Wr):
    from concourse.bass_utils import run_bass_kernel_spmd

    nc = _get_module()
    in_maps = make_in_maps(item_embedding, query_embedding, mask, Wq, bq, Wr)
    last_err = None
    for attempt in range(3):
        try:
            res = run_bass_kernel_spmd(
                nc, in_maps, core_ids=list(range(NCORES)),
                **_CACHE.get("run_kwargs", {})
            )
            break
        except Exception as e:  # transient NRT_EXEC_UNIT_UNRECOVERABLE flakes
            last_err = e
    else:
        raise last_err
    _CACHE["last_results"] = res
    return np.concatenate(
        [res.results[i]["outT"].T for i in range(NCORES)], axis=0)
